# revision 1
# baseline (speedup 1.0000x reference)
"""BinaryTreeComposer (tree-LSTM cell) Trainium2 Bass kernel.

Math (per reference):
    xi  = input @ Wi + bi                      [B, 1024]
    gl  = lh @ Wlh[g] + blh[g]   (5 gates)
    gr  = rh @ Wrh[g] + brh[g]
    pre = xi + gl + gr
    i, lf, rf, o = sigmoid(pre[0..3]); u = tanh(pre[4])
    c = i*u + lf*lc + rf*rc
    h = o*tanh(c)
    returns (c, h)

Strategy: pure data parallel over batch (16384 -> 8 x 2048), weights
replicated. Per core, 11 GEMM-units of [2048,1024]x[1024,1024] in bf16
(PSUM fp32 accumulate), fused elementwise in fp32 on DVE/ACT.

Layouts (host-packed, per core):
    xt   [MT, 128, 24, 128]  bf16  transposed activations: xt[m, p, s*8+kt, b]
                                   = src_s[m*128+b, kt*128+p], s in (input, lh, rh)
    w    [4, 128, 11, 8, 256] bf16 w[q, p, mat, kt, n] = W_mat[kt*128+p, q*256+n]
                                   mat: 0=Wi, 1..5=Wlh, 6..10=Wrh
    bias [128, 5, 1024] f32        (bi+blh[g]+brh[g]) broadcast over partitions
    lc/rc [MT, 128, 1024] f32      batch-major
Outputs c,h [MT, 128, 1024] f32 per core.
"""

import numpy as np
import ml_dtypes
from contextlib import ExitStack

B, D = 16384, 1024
NCORES = 8
P = 128
NGATES = 5
NMAT = 11
KT = 8          # k-tiles per 1024-dim source
NQ = 4          # n quarters
NB = D // NQ    # 256

_BUILD_CACHE = {}


def build(mt, repeat=1):
    """Build + compile the per-core program for mt m-tiles (batch = mt*128)."""
    import concourse.tile as tile
    from concourse import bacc, mybir

    key = (mt, repeat)
    if key in _BUILD_CACHE:
        return _BUILD_CACHE[key]

    f32 = mybir.dt.float32
    bf16 = mybir.dt.bfloat16
    Sig = mybir.ActivationFunctionType.Sigmoid
    Tanh = mybir.ActivationFunctionType.Tanh
    add = mybir.AluOpType.add
    mult = mybir.AluOpType.mult

    nc = bacc.Bacc("TRN2", target_bir_lowering=False, debug=False, num_devices=NCORES)
    xt_d = nc.dram_tensor("xt", [mt, P, 3 * KT, P], bf16, kind="ExternalInput")
    w_d = nc.dram_tensor("w", [NQ, P, NMAT, KT, NB], bf16, kind="ExternalInput")
    bias_d = nc.dram_tensor("bias", [P, NGATES, D], f32, kind="ExternalInput")
    lc_d = nc.dram_tensor("lc", [mt, P, D], f32, kind="ExternalInput")
    rc_d = nc.dram_tensor("rc", [mt, P, D], f32, kind="ExternalInput")
    c_d = nc.dram_tensor("c", [mt, P, D], f32, kind="ExternalOutput")
    h_d = nc.dram_tensor("h", [mt, P, D], f32, kind="ExternalOutput")

    with tile.TileContext(nc) as tc, ExitStack() as ctx:
        wpool = ctx.enter_context(tc.tile_pool(name="wpool", bufs=2))
        apool = ctx.enter_context(tc.tile_pool(name="apool", bufs=3))
        lpool = ctx.enter_context(tc.tile_pool(name="lpool", bufs=2))
        bpool = ctx.enter_context(tc.tile_pool(name="bpool", bufs=1))
        spool = ctx.enter_context(tc.tile_pool(name="spool", bufs=3))
        gpool = ctx.enter_context(tc.tile_pool(name="gpool", bufs=4))
        tpool = ctx.enter_context(tc.tile_pool(name="tpool", bufs=3))
        opool = ctx.enter_context(tc.tile_pool(name="opool", bufs=3))
        pspool = ctx.enter_context(tc.tile_pool(name="pspool", bufs=2, space="PSUM"))

        bias_sb = bpool.tile([P, NGATES, D], f32)
        nc.sync.dma_start(bias_sb[:], bias_d.ap())

        def body(_rep):
            for q in range(NQ):
                w_sb = wpool.tile([P, NMAT, KT, NB], bf16, tag="w")
                nc.sync.dma_start(w_sb[:], w_d.ap()[q])
                for m in range(mt):
                    act = apool.tile([P, 3 * KT, P], bf16, tag="act")
                    nc.sync.dma_start(act[:], xt_d.ap()[m])
                    lc_t = lpool.tile([P, NB], f32, tag="lc")
                    nc.sync.dma_start(lc_t[:], lc_d.ap()[m, :, q * NB:(q + 1) * NB])
                    rc_t = lpool.tile([P, NB], f32, tag="rc")
                    nc.sync.dma_start(rc_t[:], rc_d.ap()[m, :, q * NB:(q + 1) * NB])

                    # xi GEMM: K=1024 over input rows (c-slots 0..7)
                    xi_ps = pspool.tile([P, NB], f32, tag="xi", bufs=2)
                    for kt in range(KT):
                        nc.tensor.matmul(xi_ps[:], act[:, kt, :], w_sb[:, 0, kt, :],
                                         start=(kt == 0), stop=(kt == KT - 1))
                    xi_sb = spool.tile([P, NB], f32, tag="xi_sb")
                    nc.any.tensor_copy(xi_sb[:], xi_ps[:])

                    # gates, gate-major so each psum bank is consumed promptly
                    gates = []
                    for g in range(NGATES):
                        g_ps = pspool.tile([P, NB], f32, tag="gate", bufs=3)
                        for kt in range(KT):      # lh rows (c-slots 8..15)
                            nc.tensor.matmul(g_ps[:], act[:, KT + kt, :],
                                             w_sb[:, 1 + g, kt, :],
                                             start=(kt == 0), stop=False)
                        for kt in range(KT):      # rh rows (c-slots 16..23)
                            nc.tensor.matmul(g_ps[:], act[:, 2 * KT + kt, :],
                                             w_sb[:, 6 + g, kt, :],
                                             start=False, stop=(kt == KT - 1))
                        pre = tpool.tile([P, NB], f32, tag="pre", bufs=4)
                        nc.any.tensor_tensor(pre[:], g_ps[:], xi_sb[:], add)
                        nc.any.tensor_tensor(pre[:], pre[:],
                                             bias_sb[:, g, q * NB:(q + 1) * NB], add)
                        gt = gpool.tile([P, NB], f32, tag=f"gate{g}", bufs=2)
                        nc.scalar.activation(gt[:], pre[:], Sig if g < 4 else Tanh)
                        gates.append(gt)

                    i_g, lf_g, rf_g, o_g, u_g = gates
                    t1 = tpool.tile([P, NB], f32, tag="t1")
                    nc.any.tensor_tensor(t1[:], i_g[:], u_g[:], mult)
                    t2 = tpool.tile([P, NB], f32, tag="t2")
                    nc.any.tensor_tensor(t2[:], lf_g[:], lc_t[:], mult)
                    t3 = tpool.tile([P, NB], f32, tag="t3")
                    nc.any.tensor_tensor(t3[:], rf_g[:], rc_t[:], mult)
                    nc.any.tensor_tensor(t1[:], t1[:], t2[:], add)
                    c_t = opool.tile([P, NB], f32, tag="c")
                    nc.any.tensor_tensor(c_t[:], t1[:], t3[:], add)
                    nc.sync.dma_start(c_d.ap()[m, :, q * NB:(q + 1) * NB], c_t[:])
                    th = tpool.tile([P, NB], f32, tag="th")
                    nc.scalar.activation(th[:], c_t[:], Tanh)
                    h_t = opool.tile([P, NB], f32, tag="h")
                    nc.any.tensor_tensor(h_t[:], o_g[:], th[:], mult)
                    nc.sync.dma_start(h_d.ap()[m, :, q * NB:(q + 1) * NB], h_t[:])

        if repeat == 1:
            body(0)
        else:
            for r in range(repeat):
                body(r)

    nc.compile()
    _BUILD_CACHE[key] = nc
    return nc


def pack_inputs_core(x, lh, rh, lc, rc, mt):
    """Pack one core's activation inputs. x/lh/rh/lc/rc are [mt*128, 1024] f32."""
    bc = mt * P
    A = np.stack([x, lh, rh]).astype(ml_dtypes.bfloat16)      # [3, bc, 1024]
    A = A.reshape(3, mt, P, KT, P)                             # [s, m, b, kt, p]
    xt = np.ascontiguousarray(A.transpose(1, 4, 0, 3, 2))      # [m, p, s, kt, b]
    xt = xt.reshape(mt, P, 3 * KT, P)
    lc_p = np.ascontiguousarray(lc.reshape(mt, P, D))
    rc_p = np.ascontiguousarray(rc.reshape(mt, P, D))
    return xt, lc_p, rc_p


def pack_weights(Wi, bi, Wlh, blh, Wrh, brh):
    Wall = np.concatenate([Wi[None], Wlh, Wrh], axis=0).astype(ml_dtypes.bfloat16)
    # [11, 1024, 1024] -> [q, p, mat, kt, n]
    Wq = Wall.reshape(NMAT, KT, P, NQ, NB)
    w = np.ascontiguousarray(Wq.transpose(3, 2, 0, 1, 4))      # [4, 128, 11, 8, 256]
    bsum = (bi[None, :] + blh + brh).astype(np.float32)        # [5, 1024]
    bias = np.ascontiguousarray(np.broadcast_to(bsum[None], (P, NGATES, D)))
    return w, bias


def kernel(input, lc, lh, rc, rh, Wi, bi, Wlh, blh, Wrh, brh):
    from concourse.bass_utils import run_bass_kernel_spmd

    input = np.asarray(input, dtype=np.float32)
    lc = np.asarray(lc, dtype=np.float32)
    lh = np.asarray(lh, dtype=np.float32)
    rc = np.asarray(rc, dtype=np.float32)
    rh = np.asarray(rh, dtype=np.float32)

    bc = B // NCORES
    mt = bc // P
    nc = build(mt)
    w, bias = pack_weights(np.asarray(Wi), np.asarray(bi), np.asarray(Wlh),
                           np.asarray(blh), np.asarray(Wrh), np.asarray(brh))
    in_maps = []
    for c in range(NCORES):
        s = slice(c * bc, (c + 1) * bc)
        xt, lc_p, rc_p = pack_inputs_core(input[s], lh[s], rh[s], lc[s], rc[s], mt)
        in_maps.append({"xt": xt, "w": w, "bias": bias, "lc": lc_p, "rc": rc_p})

    res = run_bass_kernel_spmd(nc, in_maps, list(range(NCORES)))
    c_out = np.concatenate([res.results[i]["c"].reshape(bc, D) for i in range(NCORES)])
    h_out = np.concatenate([res.results[i]["h"].reshape(bc, D) for i in range(NCORES)])
    return c_out, h_out


# revision 5
# speedup vs baseline: 2221.0973x; 2221.0973x over previous
"""BinaryTreeComposer (tree-LSTM cell) Trainium2 Bass kernel.

Math (per reference):
    xi  = input @ Wi + bi                      [B, 1024]
    gl  = lh @ Wlh[g] + blh[g]   (5 gates)
    gr  = rh @ Wrh[g] + brh[g]
    pre = xi + gl + gr
    i, lf, rf, o = sigmoid(pre[0..3]); u = tanh(pre[4])
    c = i*u + lf*lc + rf*rc
    h = o*tanh(c)
    returns (c, h)

Strategy: pure data parallel over batch (16384 -> 8 x 2048), weights
replicated (shipped once, broadcast). Per core, 11 GEMM-units of
[2048,1024]x[1024,1024] in bf16 (PSUM fp32 accumulate, full PE rate),
fused fp32 elementwise on DVE/ACT. Device time ~600us (bf16 PE roofline
for 3.8e11 flops on 8 NeuronCore-v3).

Layouts (host-packed):
    xt   [MT, 128, 24, 128]  bf16  per core; xt[m, p, s*8+kt, b]
                                   = src_s[m*128+b, kt*128+p], s in (input, lh, rh)
    w    [4, 128, 11, 8, 256] bf16 replicated; w[q, p, mat, kt, n]
                                   = W_mat[kt*128+p, q*256+n]; mat: 0=Wi, 1..5=Wlh, 6..10=Wrh
    bias [128, 5, 1024] f32        replicated; (bi+blh[g]+brh[g]) broadcast over partitions
    lc/rc [MT, 128, 1024] f32      per core, batch-major
Outputs c,h [MT, 128, 1024] f32 per core.
"""

import numpy as np
import ml_dtypes

B, D = 16384, 1024
NCORES = 8
P = 128
NGATES = 5
NMAT = 11
KT = 8          # k-tiles per 1024-dim source
NQ = 4          # n quarters
NB = D // NQ    # 256

REPLICATED = ("w", "bias")

_BUILD_CACHE = {}
_RUNNER_CACHE = {}


def build(mt, repeat=1):
    """Build + compile the per-core program for mt m-tiles (batch = mt*128)."""
    from contextlib import ExitStack
    import concourse.tile as tile
    from concourse import bacc, mybir

    key = (mt, repeat)
    if key in _BUILD_CACHE:
        return _BUILD_CACHE[key]

    f32 = mybir.dt.float32
    bf16 = mybir.dt.bfloat16
    Sig = mybir.ActivationFunctionType.Sigmoid
    Tanh = mybir.ActivationFunctionType.Tanh
    add = mybir.AluOpType.add
    mult = mybir.AluOpType.mult

    nc = bacc.Bacc("TRN2", target_bir_lowering=False, debug=False, num_devices=NCORES)
    xt_d = nc.dram_tensor("xt", [mt, P, 3 * KT, P], bf16, kind="ExternalInput")
    w_d = nc.dram_tensor("w", [NQ, P, NMAT, KT, NB], bf16, kind="ExternalInput")
    bias_d = nc.dram_tensor("bias", [P, NGATES, D], f32, kind="ExternalInput")
    lc_d = nc.dram_tensor("lc", [mt, P, D], f32, kind="ExternalInput")
    rc_d = nc.dram_tensor("rc", [mt, P, D], f32, kind="ExternalInput")
    c_d = nc.dram_tensor("c", [mt, P, D], f32, kind="ExternalOutput")
    h_d = nc.dram_tensor("h", [mt, P, D], f32, kind="ExternalOutput")

    with tile.TileContext(nc) as tc, ExitStack() as ctx:
        wpool = ctx.enter_context(tc.tile_pool(name="wpool", bufs=2))
        apool = ctx.enter_context(tc.tile_pool(name="apool", bufs=3))
        lpool = ctx.enter_context(tc.tile_pool(name="lpool", bufs=2))
        bpool = ctx.enter_context(tc.tile_pool(name="bpool", bufs=1))
        spool = ctx.enter_context(tc.tile_pool(name="spool", bufs=3))
        gpool = ctx.enter_context(tc.tile_pool(name="gpool", bufs=4))
        tpool = ctx.enter_context(tc.tile_pool(name="tpool", bufs=3))
        opool = ctx.enter_context(tc.tile_pool(name="opool", bufs=3))
        pspool = ctx.enter_context(tc.tile_pool(name="pspool", bufs=2, space="PSUM"))

        bias_sb = bpool.tile([P, NGATES, D], f32)
        nc.sync.dma_start(bias_sb[:], bias_d.ap())

        def body(_rep):
            for q in range(NQ):
                w_sb = wpool.tile([P, NMAT, KT, NB], bf16, tag="w")
                nc.sync.dma_start(w_sb[:], w_d.ap()[q])
                for m in range(mt):
                    act = apool.tile([P, 3 * KT, P], bf16, tag="act")
                    nc.sync.dma_start(act[:], xt_d.ap()[m])
                    lc_t = lpool.tile([P, NB], f32, tag="lc")
                    nc.sync.dma_start(lc_t[:], lc_d.ap()[m, :, q * NB:(q + 1) * NB])
                    rc_t = lpool.tile([P, NB], f32, tag="rc")
                    nc.sync.dma_start(rc_t[:], rc_d.ap()[m, :, q * NB:(q + 1) * NB])

                    # xi GEMM: K=1024 over input rows (c-slots 0..7)
                    xi_ps = pspool.tile([P, NB], f32, tag="xi", bufs=2)
                    for kt in range(KT):
                        nc.tensor.matmul(xi_ps[:], act[:, kt, :], w_sb[:, 0, kt, :],
                                         start=(kt == 0), stop=(kt == KT - 1))
                    xi_sb = spool.tile([P, NB], f32, tag="xi_sb")
                    nc.any.tensor_copy(xi_sb[:], xi_ps[:])

                    # gates, gate-major so each psum bank is consumed promptly
                    gates = []
                    for g in range(NGATES):
                        g_ps = pspool.tile([P, NB], f32, tag="gate", bufs=3)
                        for kt in range(KT):      # lh rows (c-slots 8..15)
                            nc.tensor.matmul(g_ps[:], act[:, KT + kt, :],
                                             w_sb[:, 1 + g, kt, :],
                                             start=(kt == 0), stop=False)
                        for kt in range(KT):      # rh rows (c-slots 16..23)
                            nc.tensor.matmul(g_ps[:], act[:, 2 * KT + kt, :],
                                             w_sb[:, 6 + g, kt, :],
                                             start=False, stop=(kt == KT - 1))
                        pre = tpool.tile([P, NB], f32, tag="pre", bufs=4)
                        nc.any.tensor_tensor(pre[:], g_ps[:], xi_sb[:], add)
                        nc.any.tensor_tensor(pre[:], pre[:],
                                             bias_sb[:, g, q * NB:(q + 1) * NB], add)
                        gt = gpool.tile([P, NB], f32, tag=f"gate{g}", bufs=2)
                        nc.scalar.activation(gt[:], pre[:], Sig if g < 4 else Tanh)
                        gates.append(gt)

                    i_g, lf_g, rf_g, o_g, u_g = gates
                    t1 = tpool.tile([P, NB], f32, tag="t1")
                    nc.any.tensor_tensor(t1[:], i_g[:], u_g[:], mult)
                    t2 = tpool.tile([P, NB], f32, tag="t2")
                    nc.any.tensor_tensor(t2[:], lf_g[:], lc_t[:], mult)
                    t3 = tpool.tile([P, NB], f32, tag="t3")
                    nc.any.tensor_tensor(t3[:], rf_g[:], rc_t[:], mult)
                    nc.any.tensor_tensor(t1[:], t1[:], t2[:], add)
                    c_t = opool.tile([P, NB], f32, tag="c")
                    nc.any.tensor_tensor(c_t[:], t1[:], t3[:], add)
                    nc.sync.dma_start(c_d.ap()[m, :, q * NB:(q + 1) * NB], c_t[:])
                    th = tpool.tile([P, NB], f32, tag="th")
                    nc.scalar.activation(th[:], c_t[:], Tanh)
                    h_t = opool.tile([P, NB], f32, tag="h")
                    nc.any.tensor_tensor(h_t[:], o_g[:], th[:], mult)
                    nc.sync.dma_start(h_d.ap()[m, :, q * NB:(q + 1) * NB], h_t[:])

        for r in range(repeat):
            body(r)

    nc.compile()
    _BUILD_CACHE[key] = nc
    return nc


def make_runner(mt, repeat=1):
    """Memoized sharded-jit runner. Returns (fn, meta). fn(in_maps) -> results
    list of per-core dicts. Weights/bias shipped replicated (once)."""
    import jax
    from jax.sharding import Mesh, PartitionSpec, NamedSharding
    try:
        from jax import shard_map as _shard_map_mod  # jax>=0.8 path
        shard_map = _shard_map_mod
    except ImportError:
        from jax.experimental.shard_map import shard_map
    from concourse import mybir
    import concourse.bass2jax as bass2jax

    key = (mt, repeat)
    if key in _RUNNER_CACHE:
        return _RUNNER_CACHE[key]

    nc = build(mt, repeat)
    bass2jax.install_neuronx_cc_hook()
    partition_name = nc.partition_id_tensor.name if nc.partition_id_tensor else None
    in_names, out_names, out_shapes, out_dtypes = [], [], [], []
    for alloc in nc.m.functions[0].allocations:
        if not isinstance(alloc, mybir.MemoryLocationSet):
            continue
        name = alloc.memorylocations[0].name
        if alloc.kind == "ExternalInput":
            if name != partition_name:
                in_names.append(name)
        elif alloc.kind == "ExternalOutput":
            out_names.append(name)
            out_shapes.append(tuple(alloc.tensor_shape))
            out_dtypes.append(mybir.dt.np(alloc.dtype))
    out_avals = [jax.core.ShapedArray(s, d) for s, d in zip(out_shapes, out_dtypes)]
    n_params = len(in_names)
    n_outs = len(out_names)
    all_in = list(in_names) + list(out_names)
    if partition_name is not None:
        all_in.append(partition_name)
    donate = tuple(range(n_params, n_params + n_outs))

    def _body(*args):
        operands = list(args)
        if partition_name is not None:
            operands.append(bass2jax.partition_id_tensor())
        return tuple(bass2jax._bass_exec_p.bind(
            *operands, out_avals=tuple(out_avals), in_names=tuple(all_in),
            out_names=tuple(out_names), lowering_input_output_aliases=(),
            sim_require_finite=True, sim_require_nnan=True, nc=nc))

    devices = jax.devices()[:NCORES]
    mesh = Mesh(np.asarray(devices), ("core",))
    shard = PartitionSpec("core")
    repl = PartitionSpec()
    in_specs = tuple(repl if n in REPLICATED else shard for n in in_names) \
        + (shard,) * n_outs
    try:
        smapped = shard_map(_body, mesh=mesh, in_specs=in_specs,
                            out_specs=(shard,) * n_outs, check_vma=False)
    except TypeError:
        smapped = shard_map(_body, mesh=mesh, in_specs=in_specs,
                            out_specs=(shard,) * n_outs, check_rep=False)
    sharded = jax.jit(smapped, donate_argnums=donate, keep_unused=True)

    import functools
    import jax.numpy as jnp
    zero_sharding = NamedSharding(mesh, shard)

    @functools.partial(jax.jit, out_shardings=(zero_sharding,) * n_outs)
    def _make_zeros():
        return tuple(jnp.zeros((NCORES * s[0], *s[1:]), d)
                     for s, d in zip(out_shapes, out_dtypes))

    def stage(in_maps):
        dev_in = []
        for n in in_names:
            if n in REPLICATED:
                arr = jax.device_put(np.asarray(in_maps[0][n]),
                                     NamedSharding(mesh, repl))
            else:
                arr = jax.device_put(
                    np.concatenate([np.asarray(in_maps[c][n]) for c in range(NCORES)],
                                   axis=0), zero_sharding)
            dev_in.append(arr)
        jax.block_until_ready(dev_in)
        return dev_in

    def run_staged(dev_in, n_it=1):
        out = None
        for _ in range(n_it):
            out = sharded(*dev_in, *_make_zeros())
        jax.block_until_ready(out)
        return out

    def fn(in_maps, n_it=1):
        out = run_staged(stage(in_maps), n_it)
        return [
            {name: np.asarray(out[i]).reshape(NCORES, *out_shapes[i])[c]
             for i, name in enumerate(out_names)}
            for c in range(NCORES)
        ]

    fn.stage = stage
    fn.run_staged = run_staged
    _RUNNER_CACHE[key] = fn
    return fn


def pack_inputs_core(x, lh, rh, lc, rc, mt):
    """Pack one core's activation inputs. x/lh/rh/lc/rc are [mt*128, 1024] f32."""
    A = np.stack([x, lh, rh]).astype(ml_dtypes.bfloat16)      # [3, bc, 1024]
    A = A.reshape(3, mt, P, KT, P)                             # [s, m, b, kt, p]
    xt = np.ascontiguousarray(A.transpose(1, 4, 0, 3, 2))      # [m, p, s, kt, b]
    xt = xt.reshape(mt, P, 3 * KT, P)
    lc_p = np.ascontiguousarray(lc.reshape(mt, P, D))
    rc_p = np.ascontiguousarray(rc.reshape(mt, P, D))
    return xt, lc_p, rc_p


def pack_weights(Wi, bi, Wlh, blh, Wrh, brh):
    Wall = np.concatenate([Wi[None], Wlh, Wrh], axis=0).astype(ml_dtypes.bfloat16)
    # [11, 1024, 1024] -> [q, p, mat, kt, n]
    Wq = Wall.reshape(NMAT, KT, P, NQ, NB)
    w = np.ascontiguousarray(Wq.transpose(3, 2, 0, 1, 4))      # [4, 128, 11, 8, 256]
    bsum = (np.asarray(bi)[None, :] + np.asarray(blh) + np.asarray(brh)).astype(np.float32)
    bias = np.ascontiguousarray(np.broadcast_to(bsum[None], (P, NGATES, D)))
    return w, bias


def make_in_maps(input, lc, lh, rc, rh, Wi, bi, Wlh, blh, Wrh, brh):
    input = np.asarray(input, dtype=np.float32)
    lc = np.asarray(lc, dtype=np.float32)
    lh = np.asarray(lh, dtype=np.float32)
    rc = np.asarray(rc, dtype=np.float32)
    rh = np.asarray(rh, dtype=np.float32)
    bc = B // NCORES
    mt = bc // P
    w, bias = pack_weights(Wi, bi, Wlh, blh, Wrh, brh)
    in_maps = []
    for c in range(NCORES):
        s = slice(c * bc, (c + 1) * bc)
        xt, lc_p, rc_p = pack_inputs_core(input[s], lh[s], rh[s], lc[s], rc[s], mt)
        in_maps.append({"xt": xt, "w": w, "bias": bias, "lc": lc_p, "rc": rc_p})
    return in_maps, mt


def kernel(input, lc, lh, rc, rh, Wi, bi, Wlh, blh, Wrh, brh):
    in_maps, mt = make_in_maps(input, lc, lh, rc, rh, Wi, bi, Wlh, blh, Wrh, brh)
    fn = make_runner(mt)
    results = fn(in_maps)
    bc = B // NCORES
    c_out = np.concatenate([results[i]["c"].reshape(bc, D) for i in range(NCORES)])
    h_out = np.concatenate([results[i]["h"].reshape(bc, D) for i in range(NCORES)])
    return c_out, h_out
